# revision 26
# baseline (speedup 1.0000x reference)
"""Trainium2 Bass kernel for nn_Attention_39934605918652.

res[b] = W0 @ x0[b] + sum_{n=1..N-1} W2 @ tanh(W1a @ x0[b] + W1b @ x[b,n])

Key algebraic optimization: W2 does not depend on n, so
    sum_n W2 @ tanh(...) = W2 @ (sum_n tanh(...))
which removes the second big matmul (only a [B,H]x[H,F] remains).

Sharding: data-parallel over batch B=128 across 8 cores (16 batches/core),
weights replicated. No collectives.

The dominant matmul (hi = W1b @ xi, 2.1G MACs/core) runs in fp8e4 (TRN
e4m3, max +-240) with perf_mode=DoubleRow: two 128-row k-tiles are packed
per instruction, doubling MAC throughput vs bf16. W1b is pre-scaled by 64
on the host so its entries (std 0.031) sit in the fp8 normal range; the
1/64 compensation rides the ACT scale operand of the tanh. xi (std 1.0)
is cast unscaled. h0, the epilogue and phase-1 stay fp16/fp32 because
their error is amplified coherently across the N=255 summation. Measured
end-to-end rel err vs a float64 oracle: ~2.5e-3 (gate 2e-2).

DMA: descriptor issue costs ~650ns serially per queue and concurrent
transfers fair-share the 16 DMA engines, so the PE/ACT-critical stream
(w1b, xi quads, then the late-needed w2/w0) rides the sync queue in need
order while x0/w1a/oh ride the scalar queue in parallel. w1a comes in two
column halves and phase-1 is split accordingly, so the first h0 biases
(and with them the ACT stream) unblock ~2us earlier.

Per-(h,q) psum tile [128h, 4*256 (b,n)] consume paths (env-tunable mix):
  A: 4x ACT activation(tanh, bias=h0[:,b], scale=1/64) -> bf16, then
     n-segmented strided reduce on DVE (pads skipped).
  C: bias folded into the PE via a one-hot fp16 matmul (lhsT=64*h0T,
     rhs=one-hot with pad columns zeroed) appended to the accumulation
     group, then ONE 1024-col ACT tanh(scale=1/64) and a contiguous DVE
     reduce (pads are exact zeros -> tanh(0)=0 contributes nothing).
     (not available in wave 0: would stall the PE queue on w1a)
  D: DVE adds 64*h0 into psum in place (pads skipped, stay 0), then one
     1024-col ACT tanh and a contiguous reduce.
  P: like D but the four per-batch bias-adds run as tensor_scalar_add on
     the otherwise-idle Pool engine - trades idle-Pool time for ACT.
KB_FOLD=1 replaces the contiguous reduce with bf16 add-halves (eligible
for the DVE 2x/4x fast modes) + a half-size reduce.
"""

import os
import numpy as np
from contextlib import ExitStack

import concourse.bass as bass
import concourse.tile as tile
from concourse import bacc, mybir
from concourse.bass_utils import run_bass_kernel_spmd

N_CORES = 8
B, N, F, H = 128, 256, 512, 1024
BL = B // N_CORES          # 16 batches per core
NI = N - 1                 # 255 real columns per batch
NP = 256                   # padded columns per batch
NF = F // 128              # 4 f-chunks
NH = H // 128              # 8 h-tiles
QUADS = BL // 4            # 4 batch-quads; per quad psum tile [128, 4*256]
NTILE = NH * QUADS         # 32 (h,q) tiles
QC = 4 * NP                # 1024 columns per quad
W1B_SCALE = 64.0

F32 = mybir.dt.float32
BF16 = mybir.dt.bfloat16
F16 = mybir.dt.float16
FP8 = mybir.dt.float8e4

# ---- env knobs (compile keys) ----
def _cfg():
    c = int(os.environ.get("KB_C", "12"))     # tiles on path C (PE bias-mm)
    d = int(os.environ.get("KB_D", "0"))      # tiles on path D (DVE bias-add)
    p = int(os.environ.get("KB_P", "0"))      # tiles on path P (Pool bias-add)
    # PE warm-up matmuls during the DMA lead-in: the HAM clock-gates the
    # PE to 1.2GHz until ~3us of continuous work; dummies (no DMA deps)
    # cover the 7-13us window so real DR matmuls start at 2.4GHz.
    warm = int(os.environ.get("KB_WARM", "5"))
    fold = int(os.environ.get("KB_FOLD", "1"))
    assert c <= NTILE - NH and c + d + p <= NTILE
    return c, d, p, warm, fold


def _spread(k, n):
    """k of n slots True, evenly interleaved."""
    return [(i * k) // n != ((i + 1) * k) // n for i in range(n)]


def _schedule():
    c, d, p, _, _ = _cfg()
    # path C only on waves 1-3 (idx NH..NTILE-1)
    c_set = [False] * NH + _spread(c, NTILE - NH)
    paths = ["C" if c_set[i] else None for i in range(NTILE)]
    for label, cnt in (("P", p), ("D", d)):
        rest = [i for i in range(NTILE) if paths[i] is None]
        picks = _spread(min(cnt, len(rest)), len(rest))
        for j, i in enumerate(rest):
            if picks[j]:
                paths[i] = label
    for i in range(NTILE):
        if paths[i] is None:
            paths[i] = "A"
    return paths


def _build_kernel():
    nc = bacc.Bacc(
        "TRN2", target_bir_lowering=False, debug=False, num_devices=N_CORES
    )
    c, d, p, warm, fold = _cfg()
    need_h0 = c < NTILE - NH or (d + p) > 0
    need_h0s = (d + p) > 0
    need_h0T = c > 0

    # xiQ rows: [q][ (P k) ][1024] so one 2-D DMA covers a full quad
    xiQ = nc.dram_tensor(
        "xiQ", [128, QUADS * 4 * QC], FP8, kind="ExternalInput"
    ).ap()
    x0T = nc.dram_tensor("x0T", [128, NF * BL], F16, kind="ExternalInput").ap()
    w1bP = nc.dram_tensor("w1bP", [128, 4 * H], FP8, kind="ExternalInput").ap()
    w1aP = nc.dram_tensor("w1aP", [128, NF * H], F16, kind="ExternalInput").ap()
    w2P = nc.dram_tensor("w2P", [128, NH * F], F16, kind="ExternalInput").ap()
    w0P = nc.dram_tensor("w0P", [128, NF * F], F16, kind="ExternalInput").ap()
    oh = (
        nc.dram_tensor("oh", [BL, QUADS * QC], F16, kind="ExternalInput").ap()
        if need_h0T
        else None
    )
    res = nc.dram_tensor("res", [BL, F], F32, kind="ExternalOutput").ap()

    with tile.TileContext(nc) as tc:
        with ExitStack() as ctx:
            _kernel_body(
                ctx, tc, xiQ, x0T, w1bP, w1aP, w2P, w0P, oh, res,
                need_h0, need_h0s, need_h0T,
            )

    nc.compile()
    return nc


def _kernel_body(ctx, tc, xiQ, x0T, w1bP, w1aP, w2P, w0P, oh, res,
                 need_h0, need_h0s, need_h0T):
    nc = tc.nc
    Tanh = mybir.ActivationFunctionType.Tanh
    c_cnt, d_cnt, p_cnt, warm, fold = _cfg()
    paths = _schedule()

    wpool = ctx.enter_context(tc.tile_pool(name="weights", bufs=1))

    # ---- single sync queue, strict need order: concurrent queues fair-
    # share the ~330 GB/s aggregate, which delays whichever transfer gates
    # the next dependency, so one ordered queue wins.
    w1b_sb = wpool.tile([128, 4 * H], FP8, tag="w1b", name="w1b")
    nc.sync.dma_start(w1b_sb[:], w1bP[:])
    xi_sb = [None] * QUADS

    def load_xi(q):
        t = wpool.tile([128, 4 * QC], FP8, tag=f"xi_{q}", name=f"xi_{q}")
        nc.sync.dma_start(t[:], xiQ[:, q * 4 * QC : (q + 1) * 4 * QC])
        xi_sb[q] = t

    # x0 + first w1a half ahead of xi quad 0: phase-1 (which gates the ACT
    # stream) then completes right as the first psum tile lands; the PE's
    # slightly later start hides behind the warm-up dummies.
    x0_all = wpool.tile([128, NF * BL], F16, tag="x0", name="x0_all")
    nc.sync.dma_start(x0_all[:], x0T[:])
    x0_sb = [x0_all[:, f * BL : (f + 1) * BL] for f in range(NF)]
    # w1a host layout [128, (hk f hc)]: each h-half is contiguous, so each
    # half is one 2-D DMA of 2KB rows (128 descriptors).
    w1a_all = wpool.tile([128, NF * H], F16, tag="w1a", name="w1a_all")
    nc.sync.dma_start(w1a_all[:, :2048], w1aP[:, :2048])
    load_xi(0)
    nc.sync.dma_start(w1a_all[:, 2048:], w1aP[:, 2048:])

    def w1a_ap(f, h):
        off = (h // 4) * 2048 + f * 512 + (h % 4) * 128
        return w1a_all[:, off : off + 128]

    def w1a_half(f, hk):
        off = hk * 2048 + f * 512
        return w1a_all[:, off : off + 512]

    load_xi(1)
    load_xi(2)
    load_xi(3)
    oh_sb = None
    if need_h0T:
        oh_sb = wpool.tile([BL, QUADS * QC], F16, tag="oh", name="oh")
        nc.sync.dma_start(oh_sb[:], oh[:])
    w2_all = wpool.tile([128, NH * F], F16, tag="w2", name="w2_all")
    nc.sync.dma_start(w2_all[:], w2P[:])
    w2_sb = [w2_all[:, h * F : (h + 1) * F] for h in range(NH)]
    w0_all = wpool.tile([128, NF * F], F16, tag="w0", name="w0_all")
    nc.sync.dma_start(w0_all[:], w0P[:])
    w0_sb = [w0_all[:, f * F : (f + 1) * F] for f in range(NF)]

    h0_sb = h0s_sb = None
    if need_h0:
        h0_sb = [
            wpool.tile([128, BL], F32, tag=f"h0_{h}", name=f"h0_{h}")
            for h in range(NH)
        ]
    if need_h0s:
        h0s_sb = [
            wpool.tile([128, BL], F32, tag=f"h0s_{h}", name=f"h0s_{h}")
            for h in range(NH)
        ]
    h0T64_sb = None
    if need_h0T:
        h0T64_sb = wpool.tile([BL, H], F16, tag="h0T64", name="h0T64")
    S_sb = [
        wpool.tile([128, BL], F16, tag=f"S_{h}", name=f"S_{h}")
        for h in range(NH)
    ]

    # Main psum ring: slot = [128, 4*NP] f32 = 2 banks; 3 bufs = 6 banks.
    # Phase-1 h0 psums live in separate 1-bank tags so the h0 matmuls can't
    # deadlock against a main slot held by a consume that waits on h0.
    ppool = ctx.enter_context(tc.tile_pool(name="ps", bufs=3, space="PSUM"))
    phpool = ctx.enter_context(tc.tile_pool(name="ph", bufs=1, space="PSUM"))
    phTpool = ctx.enter_context(tc.tile_pool(name="phT", bufs=1, space="PSUM"))
    itpool = ctx.enter_context(tc.tile_pool(name="it", bufs=6))
    fpool = (
        ctx.enter_context(tc.tile_pool(name="fold", bufs=4)) if fold else None
    )

    # ---- Phase 0: optional PE warm-up during the DMA lead-in ----
    if warm:
        wz = wpool.tile([128, 512], F32, tag="warmz", name="warmz")
        nc.vector.memset(wz[:], 0.0)
        pw = ppool.tile([128, 512], F32, tag="ps", name="pwarm")
        for _ in range(warm):
            nc.tensor.matmul(pw[:], wz[:, :128], wz[:], start=True, stop=True)

    # ---- Phase 0b: preload the tanh ACT table during the DMA lead-in ----
    tiny = wpool.tile([128, 1], F32, tag="tiny", name="tiny")
    nc.vector.memset(tiny[:], 0.0)
    nc.scalar.activation(tiny[:], tiny[:], Tanh)

    # ---- compute phases ----
    ph_tile = [None]

    def phase1(hk):
        # h0[h, b] = sum_f W1a[h, f] * x0[b, f]  (h on partitions), for the
        # h-half hk (its w1a columns arrive in the hk-th scalar-queue DMA)
        if need_h0:
            if ph_tile[0] is None:
                ph_tile[0] = phpool.tile(
                    [128, NH * BL], F32, tag="ph", name="ph_all"
                )
            ph = ph_tile[0]
            hs = range(hk * NH // 2, (hk + 1) * NH // 2)
            for h in hs:
                for f in range(NF):
                    nc.tensor.matmul(
                        ph[:, h * BL : (h + 1) * BL],
                        w1a_ap(f, h),
                        x0_sb[f],
                        start=(f == 0),
                        stop=(f == NF - 1),
                    )
            for h in hs:
                nc.vector.tensor_copy(h0_sb[h][:], ph[:, h * BL : (h + 1) * BL])
                if need_h0s:
                    nc.vector.tensor_scalar_mul(
                        h0s_sb[h][:], ph[:, h * BL : (h + 1) * BL], W1B_SCALE
                    )
        # h0T64[b, h] = 64 * x0 @ W1a  (b on partitions, for path-C bias mm)
        if need_h0T:
            phT = phTpool.tile([BL, 512], F32, tag="phT", name=f"phT_{hk}")
            for f in range(NF):
                nc.tensor.matmul(
                    phT[:],
                    x0_sb[f],
                    w1a_half(f, hk),
                    start=(f == 0),
                    stop=(f == NF - 1),
                )
            nc.vector.tensor_scalar_mul(
                h0T64_sb[:, hk * 512 : (hk + 1) * 512], phT[:], W1B_SCALE
            )

    def dr_mms(pb, h, q, stop_here):
        # two DoubleRow matmuls per 512-col psum bank (pair P=0,1 each
        # contracting 2x128 rows); 4 matmuls per tile total
        for p in range(2):
            lhsT = (
                w1b_sb[:, p * 2 * H : (p + 1) * 2 * H]
                .rearrange("r (k h) -> r k h", k=2)[:, :, h * 128 : (h + 1) * 128]
            )
            rhs3 = xi_sb[q][
                :, p * 2 * QC : (p + 1) * 2 * QC
            ].rearrange("r (k n) -> r k n", k=2)
            for bk in range(2):
                nc.tensor.matmul(
                    pb[:, bk * 512 : (bk + 1) * 512],
                    lhsT,
                    rhs3[:, :, bk * 512 : (bk + 1) * 512],
                    start=(p == 0),
                    stop=(p == 1 and stop_here),
                    perf_mode=mybir.MatmulPerfMode.DoubleRow,
                )

    def bias_mm(pb, h, q):
        for bk in range(2):
            nc.tensor.matmul(
                pb[:, bk * 512 : (bk + 1) * 512],
                h0T64_sb[:, h * 128 : (h + 1) * 128],
                oh_sb[:, q * QC + bk * 512 : q * QC + (bk + 1) * 512],
                start=False,
                stop=True,
            )

    def consume(idx, h, q, pb):
        path = paths[idx]
        it = itpool.tile([128, QC], BF16, tag="it", name=f"it_{h}_{q}")
        if path == "A":
            for bl in range(4):
                b = q * 4 + bl
                nc.scalar.activation(
                    it[:, bl * NP : bl * NP + NI],
                    pb[:, bl * NP : bl * NP + NI],
                    Tanh,
                    bias=h0_sb[h][:, b : b + 1],
                    scale=1.0 / W1B_SCALE,
                )
        else:
            if path == "D":
                pbv = pb[:].rearrange("p (b n) -> p b n", b=4)[:, :, :NI]
                h0b = (
                    h0s_sb[h][:, q * 4 : (q + 1) * 4]
                    .unsqueeze(2)
                    .broadcast_to([128, 4, NI])
                )
                nc.vector.tensor_add(pbv, pbv, h0b)
            elif path == "P":
                for bl in range(4):
                    b = q * 4 + bl
                    sl = pb[:, bl * NP : bl * NP + NI]
                    nc.gpsimd.tensor_scalar_add(
                        sl, sl, h0s_sb[h][:, b : b + 1]
                    )
            nc.scalar.activation(it[:], pb[:], Tanh, scale=1.0 / W1B_SCALE)
        sl4 = S_sb[h][:, q * 4 : (q + 1) * 4]
        itv = it[:].rearrange("p (b n) -> p b n", b=4)
        with nc.allow_low_precision(
            reason="S accumulated in 16-bit to feed the 16-bit output matmul"
        ):
            if path == "A" or not fold:
                view = itv[:, :, :NI] if path == "A" else itv
                nc.vector.reduce_sum(sl4, view, axis=mybir.AxisListType.X)
            else:
                # pads are exact zeros: fold halves in bf16 (SBUF-to-SBUF,
                # eligible for DVE fast modes), then a half-size reduce
                fd = fpool.tile([128, 4 * 128], BF16, tag="fd", name=f"fd_{idx}")
                fdv = fd[:].rearrange("p (b n) -> p b n", b=4)
                nc.vector.tensor_add(fdv, itv[:, :, :128], itv[:, :, 128:])
                nc.vector.reduce_sum(sl4, fdv, axis=mybir.AxisListType.X)

    # ---- Phase 2: 4 waves (one batch-quad each) of 8 h-tiles ----
    # Wave-0 tiles h0/h1 fill two psum slots before phase1(0) enters the PE
    # queue (the first w1a half lands mid-wave-0); h2 covers phase1(1).
    # Consumes are emitted right after the phase1 that feeds them so the h0
    # copies precede the reduces on the DVE queue.
    deferred = []
    for q in range(QUADS):
        for h in range(NH):
            idx = q * NH + h
            pb = ppool.tile([128, QC], F32, tag="ps", name=f"pb_{h}_{q}")
            dr_mms(pb, h, q, stop_here=paths[idx] != "C")
            if paths[idx] == "C":
                bias_mm(pb, h, q)
            if q == 0 and h < 2:
                # phase1(0) (h0 for h<4) right after the first two tiles'
                # matmuls; its w1a half arrives just behind xi quad 0
                deferred.append((idx, h, q, pb))
                if h == 1:
                    phase1(0)
                    for args in deferred:
                        consume(*args)
                    deferred = None
            else:
                if q == 0 and h == 4:
                    # second w1a half lands mid-wave-0; h4-h7 biases
                    phase1(1)
                consume(idx, h, q, pb)

    # ---- Phase 3: res[b, g] = sum_h S[h,b] W2T[h,g] + sum_f x0T[f,b] W0T[f,g]
    po = ppool.tile([BL, F], F32, tag="ps", name="po")
    for f in range(NF):
        nc.tensor.matmul(
            po[:], x0_sb[f], w0_sb[f], start=(f == 0), stop=False
        )
    for h in range(NH):
        nc.tensor.matmul(
            po[:], S_sb[h][:], w2_sb[h], start=False, stop=(h == NH - 1)
        )
    rt = itpool.tile([BL, F], F32, tag="rt", name="rt")
    nc.vector.tensor_copy(rt[:], po[:])
    nc.sync.dma_start(res[:], rt[:])


_NC_CACHE = {}


def _get_nc():
    key = ("v50",) + _cfg()
    if key not in _NC_CACHE:
        _NC_CACHE[key] = _build_kernel()
    return _NC_CACHE[key]


def _to_fp8(a):
    import ml_dtypes

    return np.clip(a, -240.0, 240.0).astype(ml_dtypes.float8_e4m3)


def _pk_rows(M):
    """[512, X] f-major -> [128, (P k), X]: row p holds k-tile (2P+k) row p."""
    X = M.shape[1]
    return M.reshape(2, 2, 128, X).transpose(2, 0, 1, 3)  # [128, P, k, X]


def _make_in_maps(x, W1, W2, W0):
    c_cnt, d_cnt, p_cnt, warm, fold = _cfg()
    need_h0T = c_cnt > 0
    x = np.ascontiguousarray(np.asarray(x, dtype=np.float32))
    W1 = np.asarray(W1, dtype=np.float32)
    W2 = np.asarray(W2, dtype=np.float32)
    W0 = np.asarray(W0, dtype=np.float32)

    # [p, hk, f, hc]: each h-half of w1a contiguous per row (one DMA each)
    w1aP = np.ascontiguousarray(
        W1[:, :F].T.reshape(NF, 128, 2, 512)
        .transpose(1, 2, 0, 3)
        .reshape(128, NF * H)
    ).astype(np.float16)
    w1bP = _to_fp8(
        _pk_rows(np.ascontiguousarray(W1[:, F:].T) * W1B_SCALE).reshape(128, 4 * H)
    )
    w2P = np.ascontiguousarray(
        W2.T.reshape(NH, 128, F).transpose(1, 0, 2).reshape(128, NH * F)
    ).astype(np.float16)
    w0P = np.ascontiguousarray(
        W0.T.reshape(NF, 128, F).transpose(1, 0, 2).reshape(128, NF * F)
    ).astype(np.float16)
    if need_h0T:
        ohm = np.zeros((BL, QUADS, 4, NP), dtype=np.float16)
        for q in range(QUADS):
            for bl in range(4):
                ohm[q * 4 + bl, q, bl, :NI] = 1.0
        ohm = np.ascontiguousarray(ohm.reshape(BL, QUADS * QC))

    in_maps = []
    for i in range(N_CORES):
        xc = x[i * BL : (i + 1) * BL]               # [BL, N, F]
        x0T = np.ascontiguousarray(
            xc[:, 0, :].T.reshape(NF, 128, BL).transpose(1, 0, 2).reshape(128, NF * BL)
        ).astype(np.float16)
        pad = np.zeros((BL, NP, F), dtype=np.float32)
        pad[:, :NI, :] = xc[:, 1:, :]
        xiT = np.ascontiguousarray(pad.reshape(BL * NP, F).T)     # [F, BL*NP]
        v = _pk_rows(xiT)                                         # [128, P, k, BL*NP]
        # -> [128, q, P, k, 1024] -> [128, q*(Pk)*1024]
        xiQ = np.ascontiguousarray(
            v.reshape(128, 2, 2, QUADS, QC)
            .transpose(0, 3, 1, 2, 4)
            .reshape(128, QUADS * 4 * QC)
        )
        m = {
            "xiQ": _to_fp8(xiQ),
            "x0T": x0T,
            "w1bP": w1bP,
            "w1aP": w1aP,
            "w2P": w2P,
            "w0P": w0P,
        }
        if need_h0T:
            m["oh"] = ohm
        in_maps.append(m)
    return in_maps


def _gather(results):
    out = np.empty((B, F), dtype=np.float32)
    for i in range(N_CORES):
        out[i * BL : (i + 1) * BL] = results[i]["res"]
    return out


def kernel(x, W1, W2, W0):
    nc = _get_nc()
    in_maps = _make_in_maps(x, W1, W2, W0)
    res = run_bass_kernel_spmd(nc, in_maps, list(range(N_CORES)))
    return _gather(res.results)


def kernel_profiled(x, W1, W2, W0, **trace_kwargs):
    """Like kernel() but with NTFF profiling; returns (out, exec_time_ns)."""
    nc = _get_nc()
    in_maps = _make_in_maps(x, W1, W2, W0)
    res = run_bass_kernel_spmd(
        nc, in_maps, list(range(N_CORES)), trace=True, **trace_kwargs
    )
    return _gather(res.results), res.exec_time_ns


# revision 28
# speedup vs baseline: 1.0046x; 1.0046x over previous
"""Trainium2 Bass kernel for nn_Attention_39934605918652.

res[b] = W0 @ x0[b] + sum_{n=1..N-1} W2 @ tanh(W1a @ x0[b] + W1b @ x[b,n])

Key algebraic optimization: W2 does not depend on n, so
    sum_n W2 @ tanh(...) = W2 @ (sum_n tanh(...))
which removes the second big matmul (only a [B,H]x[H,F] remains).

Sharding: data-parallel over batch B=128 across 8 cores (16 batches/core),
weights replicated. No collectives.

The dominant matmul (hi = W1b @ xi, 2.1G MACs/core) runs in fp8e4 (TRN
e4m3, max +-240) with perf_mode=DoubleRow: two 128-row k-tiles are packed
per instruction, doubling MAC throughput vs bf16. W1b is pre-scaled by 64
on the host so its entries (std 0.031) sit in the fp8 normal range; the
1/64 compensation rides the ACT scale operand of the tanh. xi (std 1.0)
is cast unscaled. h0, the epilogue and phase-1 stay fp16/fp32 because
their error is amplified coherently across the N=255 summation. Measured
end-to-end rel err vs a float64 oracle: ~2.5e-3 (gate 2e-2).

DMA: descriptor issue costs ~650ns serially per queue and concurrent
transfers fair-share the 16 DMA engines, so the PE/ACT-critical stream
(w1b, xi quads, then the late-needed w2/w0) rides the sync queue in need
order while x0/w1a/oh ride the scalar queue in parallel. w1a comes in two
column halves and phase-1 is split accordingly, so the first h0 biases
(and with them the ACT stream) unblock ~2us earlier.

Per-(h,q) psum tile [128h, 4*256 (b,n)] consume paths (env-tunable mix):
  A: 4x ACT activation(tanh, bias=h0[:,b], scale=1/64) -> bf16, then
     n-segmented strided reduce on DVE (pads skipped).
  C: bias folded into the PE via a one-hot fp16 matmul (lhsT=64*h0T,
     rhs=one-hot with pad columns zeroed) appended to the accumulation
     group, then ONE 1024-col ACT tanh(scale=1/64) and a contiguous DVE
     reduce (pads are exact zeros -> tanh(0)=0 contributes nothing).
     (not available in wave 0: would stall the PE queue on w1a)
  D: DVE adds 64*h0 into psum in place (pads skipped, stay 0), then one
     1024-col ACT tanh and a contiguous reduce.
  P: like D but the four per-batch bias-adds run as tensor_scalar_add on
     the otherwise-idle Pool engine - trades idle-Pool time for ACT.
KB_FOLD=1 replaces the contiguous reduce with bf16 add-halves (eligible
for the DVE 2x/4x fast modes) + a half-size reduce.
"""

import os
import numpy as np
from contextlib import ExitStack

import concourse.bass as bass
import concourse.tile as tile
from concourse import bacc, mybir
from concourse.bass_utils import run_bass_kernel_spmd

N_CORES = 8
B, N, F, H = 128, 256, 512, 1024
BL = B // N_CORES          # 16 batches per core
NI = N - 1                 # 255 real columns per batch
NP = 256                   # padded columns per batch
NF = F // 128              # 4 f-chunks
NH = H // 128              # 8 h-tiles
QUADS = BL // 4            # 4 batch-quads; per quad psum tile [128, 4*256]
NTILE = NH * QUADS         # 32 (h,q) tiles
QC = 4 * NP                # 1024 columns per quad
W1B_SCALE = 64.0

F32 = mybir.dt.float32
BF16 = mybir.dt.bfloat16
F16 = mybir.dt.float16
FP8 = mybir.dt.float8e4

# ---- env knobs (compile keys) ----
def _cfg():
    c = int(os.environ.get("KB_C", "14"))     # tiles on path C (PE bias-mm)
    d = int(os.environ.get("KB_D", "0"))      # tiles on path D (DVE bias-add)
    p = int(os.environ.get("KB_P", "0"))      # tiles on path P (Pool bias-add)
    # PE warm-up matmuls during the DMA lead-in: the HAM clock-gates the
    # PE to 1.2GHz until ~3us of continuous work; dummies (no DMA deps)
    # cover the 7-13us window so real DR matmuls start at 2.4GHz.
    warm = int(os.environ.get("KB_WARM", "4"))
    fold = int(os.environ.get("KB_FOLD", "1"))
    assert c <= NTILE - NH and c + d + p <= NTILE
    return c, d, p, warm, fold


def _spread(k, n):
    """k of n slots True, evenly interleaved."""
    return [(i * k) // n != ((i + 1) * k) // n for i in range(n)]


def _schedule():
    c, d, p, _, _ = _cfg()
    # path C only on waves 1-3 (idx NH..NTILE-1)
    c_set = [False] * NH + _spread(c, NTILE - NH)
    paths = ["C" if c_set[i] else None for i in range(NTILE)]
    for label, cnt in (("P", p), ("D", d)):
        rest = [i for i in range(NTILE) if paths[i] is None]
        picks = _spread(min(cnt, len(rest)), len(rest))
        for j, i in enumerate(rest):
            if picks[j]:
                paths[i] = label
    for i in range(NTILE):
        if paths[i] is None:
            paths[i] = "A"
    return paths


def _build_kernel():
    nc = bacc.Bacc(
        "TRN2", target_bir_lowering=False, debug=False, num_devices=N_CORES
    )
    c, d, p, warm, fold = _cfg()
    need_h0 = c < NTILE - NH or (d + p) > 0
    need_h0s = (d + p) > 0
    need_h0T = c > 0

    # xiQ rows: [q][ (P k) ][1024] so one 2-D DMA covers a full quad
    xiQ = nc.dram_tensor(
        "xiQ", [128, QUADS * 4 * QC], FP8, kind="ExternalInput"
    ).ap()
    x0T = nc.dram_tensor("x0T", [128, NF * BL], F16, kind="ExternalInput").ap()
    w1bP = nc.dram_tensor("w1bP", [128, 4 * H], FP8, kind="ExternalInput").ap()
    w1aP = nc.dram_tensor("w1aP", [128, NF * H], F16, kind="ExternalInput").ap()
    w2P = nc.dram_tensor("w2P", [128, NH * F], F16, kind="ExternalInput").ap()
    w0P = nc.dram_tensor("w0P", [128, NF * F], F16, kind="ExternalInput").ap()
    oh = (
        nc.dram_tensor("oh", [BL, QUADS * QC], F16, kind="ExternalInput").ap()
        if need_h0T
        else None
    )
    res = nc.dram_tensor("res", [BL, F], F32, kind="ExternalOutput").ap()

    with tile.TileContext(nc) as tc:
        with ExitStack() as ctx:
            _kernel_body(
                ctx, tc, xiQ, x0T, w1bP, w1aP, w2P, w0P, oh, res,
                need_h0, need_h0s, need_h0T,
            )

    nc.compile()
    return nc


def _kernel_body(ctx, tc, xiQ, x0T, w1bP, w1aP, w2P, w0P, oh, res,
                 need_h0, need_h0s, need_h0T):
    nc = tc.nc
    Tanh = mybir.ActivationFunctionType.Tanh
    c_cnt, d_cnt, p_cnt, warm, fold = _cfg()
    paths = _schedule()

    wpool = ctx.enter_context(tc.tile_pool(name="weights", bufs=1))

    # ---- single sync queue, strict need order: concurrent queues fair-
    # share the ~330 GB/s aggregate, which delays whichever transfer gates
    # the next dependency, so one ordered queue wins.
    w1b_sb = wpool.tile([128, 4 * H], FP8, tag="w1b", name="w1b")
    nc.sync.dma_start(w1b_sb[:], w1bP[:])
    xi_sb = [None] * QUADS

    def load_xi(q):
        t = wpool.tile([128, 4 * QC], FP8, tag=f"xi_{q}", name=f"xi_{q}")
        nc.sync.dma_start(t[:], xiQ[:, q * 4 * QC : (q + 1) * 4 * QC])
        xi_sb[q] = t

    load_xi(0)
    x0_all = wpool.tile([128, NF * BL], F16, tag="x0", name="x0_all")
    nc.sync.dma_start(x0_all[:], x0T[:])
    x0_sb = [x0_all[:, f * BL : (f + 1) * BL] for f in range(NF)]
    # w1a host layout [128, (hk f hc)]: each h-half is contiguous, so each
    # half is one 2-D DMA of 2KB rows (128 descriptors).
    w1a_all = wpool.tile([128, NF * H], F16, tag="w1a", name="w1a_all")
    for hk in range(2):
        nc.sync.dma_start(
            w1a_all[:, hk * 2048 : (hk + 1) * 2048],
            w1aP[:, hk * 2048 : (hk + 1) * 2048],
        )

    def w1a_ap(f, h):
        off = (h // 4) * 2048 + f * 512 + (h % 4) * 128
        return w1a_all[:, off : off + 128]

    def w1a_half(f, hk):
        off = hk * 2048 + f * 512
        return w1a_all[:, off : off + 512]

    load_xi(1)
    load_xi(2)
    load_xi(3)
    oh_sb = None
    if need_h0T:
        oh_sb = wpool.tile([BL, QUADS * QC], F16, tag="oh", name="oh")
        nc.sync.dma_start(oh_sb[:], oh[:])
    w2_all = wpool.tile([128, NH * F], F16, tag="w2", name="w2_all")
    nc.sync.dma_start(w2_all[:], w2P[:])
    w2_sb = [w2_all[:, h * F : (h + 1) * F] for h in range(NH)]
    w0_all = wpool.tile([128, NF * F], F16, tag="w0", name="w0_all")
    nc.sync.dma_start(w0_all[:], w0P[:])
    w0_sb = [w0_all[:, f * F : (f + 1) * F] for f in range(NF)]

    h0_sb = h0s_sb = None
    if need_h0:
        h0_sb = [
            wpool.tile([128, BL], F32, tag=f"h0_{h}", name=f"h0_{h}")
            for h in range(NH)
        ]
    if need_h0s:
        h0s_sb = [
            wpool.tile([128, BL], F32, tag=f"h0s_{h}", name=f"h0s_{h}")
            for h in range(NH)
        ]
    h0T64_sb = None
    if need_h0T:
        h0T64_sb = wpool.tile([BL, H], F16, tag="h0T64", name="h0T64")
    S_sb = [
        wpool.tile([128, BL], F16, tag=f"S_{h}", name=f"S_{h}")
        for h in range(NH)
    ]

    # Main psum ring: slot = [128, 4*NP] f32 = 2 banks; 3 bufs = 6 banks.
    # Phase-1 h0 psums live in separate 1-bank tags so the h0 matmuls can't
    # deadlock against a main slot held by a consume that waits on h0.
    ppool = ctx.enter_context(tc.tile_pool(name="ps", bufs=3, space="PSUM"))
    phpool = ctx.enter_context(tc.tile_pool(name="ph", bufs=1, space="PSUM"))
    phTpool = ctx.enter_context(tc.tile_pool(name="phT", bufs=1, space="PSUM"))
    itpool = ctx.enter_context(tc.tile_pool(name="it", bufs=6))
    fpool = (
        ctx.enter_context(tc.tile_pool(name="fold", bufs=4)) if fold else None
    )

    # ---- Phase 0: optional PE warm-up during the DMA lead-in ----
    if warm:
        wz = wpool.tile([128, 512], F32, tag="warmz", name="warmz")
        nc.vector.memset(wz[:], 0.0)
        pw = ppool.tile([128, 512], F32, tag="ps", name="pwarm")
        for _ in range(warm):
            nc.tensor.matmul(pw[:], wz[:, :128], wz[:], start=True, stop=True)

    # ---- Phase 0b: preload the tanh ACT table during the DMA lead-in ----
    tiny = wpool.tile([128, 1], F32, tag="tiny", name="tiny")
    nc.vector.memset(tiny[:], 0.0)
    nc.scalar.activation(tiny[:], tiny[:], Tanh)

    # ---- compute phases ----
    ph_tile = [None]

    def phase1(hk):
        # h0[h, b] = sum_f W1a[h, f] * x0[b, f]  (h on partitions), for the
        # h-half hk (its w1a columns arrive in the hk-th scalar-queue DMA)
        if need_h0:
            if ph_tile[0] is None:
                ph_tile[0] = phpool.tile(
                    [128, NH * BL], F32, tag="ph", name="ph_all"
                )
            ph = ph_tile[0]
            hs = range(hk * NH // 2, (hk + 1) * NH // 2)
            for h in hs:
                for f in range(NF):
                    nc.tensor.matmul(
                        ph[:, h * BL : (h + 1) * BL],
                        w1a_ap(f, h),
                        x0_sb[f],
                        start=(f == 0),
                        stop=(f == NF - 1),
                    )
            for h in hs:
                nc.vector.tensor_copy(h0_sb[h][:], ph[:, h * BL : (h + 1) * BL])
                if need_h0s:
                    nc.vector.tensor_scalar_mul(
                        h0s_sb[h][:], ph[:, h * BL : (h + 1) * BL], W1B_SCALE
                    )
        # h0T64[b, h] = 64 * x0 @ W1a  (b on partitions, for path-C bias mm)
        if need_h0T:
            phT = phTpool.tile([BL, 512], F32, tag="phT", name=f"phT_{hk}")
            for f in range(NF):
                nc.tensor.matmul(
                    phT[:],
                    x0_sb[f],
                    w1a_half(f, hk),
                    start=(f == 0),
                    stop=(f == NF - 1),
                )
            nc.vector.tensor_scalar_mul(
                h0T64_sb[:, hk * 512 : (hk + 1) * 512], phT[:], W1B_SCALE
            )

    def dr_mms(pb, h, q, stop_here):
        # two DoubleRow matmuls per 512-col psum bank (pair P=0,1 each
        # contracting 2x128 rows); 4 matmuls per tile total
        for p in range(2):
            lhsT = (
                w1b_sb[:, p * 2 * H : (p + 1) * 2 * H]
                .rearrange("r (k h) -> r k h", k=2)[:, :, h * 128 : (h + 1) * 128]
            )
            rhs3 = xi_sb[q][
                :, p * 2 * QC : (p + 1) * 2 * QC
            ].rearrange("r (k n) -> r k n", k=2)
            for bk in range(2):
                nc.tensor.matmul(
                    pb[:, bk * 512 : (bk + 1) * 512],
                    lhsT,
                    rhs3[:, :, bk * 512 : (bk + 1) * 512],
                    start=(p == 0),
                    stop=(p == 1 and stop_here),
                    perf_mode=mybir.MatmulPerfMode.DoubleRow,
                )

    def bias_mm(pb, h, q):
        for bk in range(2):
            nc.tensor.matmul(
                pb[:, bk * 512 : (bk + 1) * 512],
                h0T64_sb[:, h * 128 : (h + 1) * 128],
                oh_sb[:, q * QC + bk * 512 : q * QC + (bk + 1) * 512],
                start=False,
                stop=True,
            )

    def consume(idx, h, q, pb):
        path = paths[idx]
        it = itpool.tile([128, QC], BF16, tag="it", name=f"it_{h}_{q}")
        if path == "A":
            for bl in range(4):
                b = q * 4 + bl
                nc.scalar.activation(
                    it[:, bl * NP : bl * NP + NI],
                    pb[:, bl * NP : bl * NP + NI],
                    Tanh,
                    bias=h0_sb[h][:, b : b + 1],
                    scale=1.0 / W1B_SCALE,
                )
        else:
            if path == "D":
                pbv = pb[:].rearrange("p (b n) -> p b n", b=4)[:, :, :NI]
                h0b = (
                    h0s_sb[h][:, q * 4 : (q + 1) * 4]
                    .unsqueeze(2)
                    .broadcast_to([128, 4, NI])
                )
                nc.vector.tensor_add(pbv, pbv, h0b)
            elif path == "P":
                for bl in range(4):
                    b = q * 4 + bl
                    sl = pb[:, bl * NP : bl * NP + NI]
                    nc.gpsimd.tensor_scalar_add(
                        sl, sl, h0s_sb[h][:, b : b + 1]
                    )
            nc.scalar.activation(it[:], pb[:], Tanh, scale=1.0 / W1B_SCALE)
        sl4 = S_sb[h][:, q * 4 : (q + 1) * 4]
        itv = it[:].rearrange("p (b n) -> p b n", b=4)
        with nc.allow_low_precision(
            reason="S accumulated in 16-bit to feed the 16-bit output matmul"
        ):
            if path == "A" or not fold:
                view = itv[:, :, :NI] if path == "A" else itv
                nc.vector.reduce_sum(sl4, view, axis=mybir.AxisListType.X)
            else:
                # pads are exact zeros: fold halves in bf16 (SBUF-to-SBUF,
                # eligible for DVE fast modes), then a half-size reduce
                fd = fpool.tile([128, 4 * 128], BF16, tag="fd", name=f"fd_{idx}")
                fdv = fd[:].rearrange("p (b n) -> p b n", b=4)
                nc.vector.tensor_add(fdv, itv[:, :, :128], itv[:, :, 128:])
                nc.vector.reduce_sum(sl4, fdv, axis=mybir.AxisListType.X)

    # ---- Phase 2: 4 waves (one batch-quad each) of 8 h-tiles ----
    # Wave-0 tiles h0/h1 fill two psum slots before phase1(0) enters the PE
    # queue (the first w1a half lands mid-wave-0); h2 covers phase1(1).
    # Consumes are emitted right after the phase1 that feeds them so the h0
    # copies precede the reduces on the DVE queue.
    deferred = []
    for q in range(QUADS):
        for h in range(NH):
            idx = q * NH + h
            pb = ppool.tile([128, QC], F32, tag="ps", name=f"pb_{h}_{q}")
            dr_mms(pb, h, q, stop_here=paths[idx] != "C")
            if paths[idx] == "C":
                bias_mm(pb, h, q)
            if q == 0 and h < 2:
                # phase1(0) (h0 for h<4) right after the first two tiles'
                # matmuls; its w1a half arrives just behind xi quad 0
                deferred.append((idx, h, q, pb))
                if h == 1:
                    phase1(0)
                    for args in deferred:
                        consume(*args)
                    deferred = None
            else:
                if q == 0 and h == 4:
                    # second w1a half lands mid-wave-0; h4-h7 biases
                    phase1(1)
                consume(idx, h, q, pb)

    # ---- Phase 3: res[b, g] = sum_h S[h,b] W2T[h,g] + sum_f x0T[f,b] W0T[f,g]
    po = ppool.tile([BL, F], F32, tag="ps", name="po")
    for f in range(NF):
        nc.tensor.matmul(
            po[:], x0_sb[f], w0_sb[f], start=(f == 0), stop=False
        )
    for h in range(NH):
        nc.tensor.matmul(
            po[:], S_sb[h][:], w2_sb[h], start=False, stop=(h == NH - 1)
        )
    rt = itpool.tile([BL, F], F32, tag="rt", name="rt")
    nc.vector.tensor_copy(rt[:], po[:])
    nc.sync.dma_start(res[:], rt[:])


_NC_CACHE = {}


def _get_nc():
    key = ("v50",) + _cfg()
    if key not in _NC_CACHE:
        _NC_CACHE[key] = _build_kernel()
    return _NC_CACHE[key]


def _to_fp8(a):
    import ml_dtypes

    return np.clip(a, -240.0, 240.0).astype(ml_dtypes.float8_e4m3)


def _pk_rows(M):
    """[512, X] f-major -> [128, (P k), X]: row p holds k-tile (2P+k) row p."""
    X = M.shape[1]
    return M.reshape(2, 2, 128, X).transpose(2, 0, 1, 3)  # [128, P, k, X]


def _make_in_maps(x, W1, W2, W0):
    c_cnt, d_cnt, p_cnt, warm, fold = _cfg()
    need_h0T = c_cnt > 0
    x = np.ascontiguousarray(np.asarray(x, dtype=np.float32))
    W1 = np.asarray(W1, dtype=np.float32)
    W2 = np.asarray(W2, dtype=np.float32)
    W0 = np.asarray(W0, dtype=np.float32)

    # [p, hk, f, hc]: each h-half of w1a contiguous per row (one DMA each)
    w1aP = np.ascontiguousarray(
        W1[:, :F].T.reshape(NF, 128, 2, 512)
        .transpose(1, 2, 0, 3)
        .reshape(128, NF * H)
    ).astype(np.float16)
    w1bP = _to_fp8(
        _pk_rows(np.ascontiguousarray(W1[:, F:].T) * W1B_SCALE).reshape(128, 4 * H)
    )
    w2P = np.ascontiguousarray(
        W2.T.reshape(NH, 128, F).transpose(1, 0, 2).reshape(128, NH * F)
    ).astype(np.float16)
    w0P = np.ascontiguousarray(
        W0.T.reshape(NF, 128, F).transpose(1, 0, 2).reshape(128, NF * F)
    ).astype(np.float16)
    if need_h0T:
        ohm = np.zeros((BL, QUADS, 4, NP), dtype=np.float16)
        for q in range(QUADS):
            for bl in range(4):
                ohm[q * 4 + bl, q, bl, :NI] = 1.0
        ohm = np.ascontiguousarray(ohm.reshape(BL, QUADS * QC))

    in_maps = []
    for i in range(N_CORES):
        xc = x[i * BL : (i + 1) * BL]               # [BL, N, F]
        x0T = np.ascontiguousarray(
            xc[:, 0, :].T.reshape(NF, 128, BL).transpose(1, 0, 2).reshape(128, NF * BL)
        ).astype(np.float16)
        pad = np.zeros((BL, NP, F), dtype=np.float32)
        pad[:, :NI, :] = xc[:, 1:, :]
        xiT = np.ascontiguousarray(pad.reshape(BL * NP, F).T)     # [F, BL*NP]
        v = _pk_rows(xiT)                                         # [128, P, k, BL*NP]
        # -> [128, q, P, k, 1024] -> [128, q*(Pk)*1024]
        xiQ = np.ascontiguousarray(
            v.reshape(128, 2, 2, QUADS, QC)
            .transpose(0, 3, 1, 2, 4)
            .reshape(128, QUADS * 4 * QC)
        )
        m = {
            "xiQ": _to_fp8(xiQ),
            "x0T": x0T,
            "w1bP": w1bP,
            "w1aP": w1aP,
            "w2P": w2P,
            "w0P": w0P,
        }
        if need_h0T:
            m["oh"] = ohm
        in_maps.append(m)
    return in_maps


def _gather(results):
    out = np.empty((B, F), dtype=np.float32)
    for i in range(N_CORES):
        out[i * BL : (i + 1) * BL] = results[i]["res"]
    return out


def kernel(x, W1, W2, W0):
    nc = _get_nc()
    in_maps = _make_in_maps(x, W1, W2, W0)
    res = run_bass_kernel_spmd(nc, in_maps, list(range(N_CORES)))
    return _gather(res.results)


def kernel_profiled(x, W1, W2, W0, **trace_kwargs):
    """Like kernel() but with NTFF profiling; returns (out, exec_time_ns)."""
    nc = _get_nc()
    in_maps = _make_in_maps(x, W1, W2, W0)
    res = run_bass_kernel_spmd(
        nc, in_maps, list(range(N_CORES)), trace=True, **trace_kwargs
    )
    return _gather(res.results), res.exec_time_ns
